# revision 11
# baseline (speedup 1.0000x reference)
"""LSG (local-sparse-global) block-local self-attention for Trainium2.

Problem: n=2, h=16, t=4096, d=64, block=128. Each query block attends to a
3-block local key window (1-block halo each side) plus a global BOS token
slot; the BOS query (position 0) attends to everything.

Strategy (8 NeuronCores, batch*head = 32 sharded 4 per core, as 2 "bh
pairs" per core). The PE clock holds at 1.2 GHz in this environment, so
everything is built to keep the 128x128 array fully packed:

  - QK^T: the two bh of a pair ride the two PE row groups (contraction
    d=64 each) concurrently: lhsT = kT(bh0) on partitions 0-63 and
    kT(bh1) on 64-127, each against its own 384-wide query union, into
    separate PSUM banks. Full-array utilization, no q duplication.
  - softmax has no running max: p = exp(s/8). Scores/8 are ~N(0,1) so
    plain exp stays in fp32/bf16 range and constant bias cancels after
    the host-side normalization.
  - exp is the scalar-engine's job alone in HW; to unbottleneck it, key
    blocks alternate between ACT (exact spline Exp) and DVE using a
    single-pass Schraudolph bit-trick: i16 = round(s*23.083 + b) stored
    via an int16 bitcast IS the bf16 encoding of exp(s/8) (+-3% sawtooth
    which cancels to ~1e-2 absmax after normalization).
  - PV runs transposed so M=128 (full array): out[q, 0:65] accumulates
    pt_j^T @ [v_j | 1] with pt (keys x queries) as the stationary
    operand and the 65-wide v-augmented block as the moving operand.
    Accumulation groups live in per-4-query-block "generation" PSUM
    banks using the per-element has_written semantics: one start=True on
    the first write of a bank generation, everything else accumulates or
    first-writes naturally.
  - outputs (64 out dims + softmax sum) leave as bf16; host divides by
    sums, adds the BOS-token key slot for query blocks >= 2 (blocks 0/1
    already have key 0 in-window, matching the reference's global-slot
    semantics), and computes the single BOS query row (~0.5% of FLOPs).
"""

import sys

import numpy as np
import ml_dtypes

try:  # concourse (bass) ships in the trn_rl repo, not on the default path
    import concourse.bass  # noqa: F401
except ImportError:
    for _p in ("/opt/trn_rl_repo", "/root/.axon_site/_ro/trn_rl_repo"):
        if _p not in sys.path:
            sys.path.insert(0, _p)

N, H, T, D = 2, 16, 4096, 64
BLOCK = 128
NB = T // BLOCK            # 32 key/query blocks
BH = N * H                 # 32 batch*head pairs
NCORES = 8
BH_PER_CORE = BH // NCORES  # 4
NPAIR = BH_PER_CORE // 2    # 2 bh-pairs per core
SCALE = 1.0 / 8.0           # 1/sqrt(64)
VW = D + 1                  # 65: v columns + ones column
GSTRIDE = 66                # psum column stride per query-block group (8B align)

# exp split: key blocks in DVE_JS run the DVE Schraudolph exp, others ACT
DVE_JS = frozenset(range(0, 32, 2))
SCH_C = 0.02
SCH_A = float(128.0 * np.log2(np.e) * SCALE)     # 23.083
SCH_B = float(16256.0 - SCH_C * 128.0)

_BF16 = ml_dtypes.bfloat16

_CACHE = {}


def _build_bass():
    import concourse.bacc as bacc
    import concourse.mybir as mybir
    import concourse.tile as tile

    bf16 = mybir.dt.bfloat16
    i16 = mybir.dt.int16
    f32 = mybir.dt.float32

    nc = bacc.Bacc(None, target_bir_lowering=False)
    # qt/kt: [pair, 128, T]: partitions 0-63 = bh even (d-major transpose),
    # 64-127 = bh odd.
    qt = nc.declare_dram_parameter("qt", [NPAIR, 128, T], bf16, isOutput=False)
    kt = nc.declare_dram_parameter("kt", [NPAIR, 128, T], bf16, isOutput=False)
    # va: [pair, bh_slot, 128, NB*65]: per key block j, cols 65j..65j+64 hold
    # [v[128j + p, :], 1.0] on partition p.
    va = nc.declare_dram_parameter(
        "va", [NPAIR, 2, 128, NB * VW], bf16, isOutput=False
    )
    # out: [pair, bh_slot, 4, 128, 520]: tile t holds query blocks 8t..8t+7
    # as 8 x 65 col groups; partition dim = query within block.
    out = nc.declare_dram_parameter(
        "out", [NPAIR, 2, 4, 128, 2 * 4 * VW], bf16, isOutput=True
    )

    AH = 17 * BLOCK     # 2176: first-half tile width (blocks 0-16)

    with tile.TileContext(nc) as tc:
        with (
            tc.tile_pool(name="cst", bufs=1) as cst,
            tc.tile_pool(name="sbq", bufs=2) as sbq,
            tc.tile_pool(name="sbk", bufs=2) as sbk,
            tc.tile_pool(name="sbv", bufs=2) as sbv,
            tc.tile_pool(name="sbp", bufs=5) as sbp,
            tc.tile_pool(name="sbo", bufs=2) as sbo,
            tc.tile_pool(name="psS", bufs=2, space="PSUM") as psS,
            tc.tile_pool(name="psG", bufs=2, space="PSUM") as psG,
        ):
            bias_tile = cst.tile([128, 1], f32, tag="bias")
            nc.vector.memset(bias_tile, 0.0)
            # Touch Exp from ACT once: loads the exp table set during warmup
            # and keeps later Exp ops from each carrying a cross-engine wait.
            warm = cst.tile([128, 1], f32, tag="warm")
            nc.scalar.activation(
                out=warm,
                in_=bias_tile,
                func=mybir.ActivationFunctionType.Exp,
                bias=0.0,
                scale=1.0,
            )
            # PE warmup: back-to-back matmuls so the PE pipeline is live
            # while the first DMA loads run.
            wsrc = cst.tile([64, 512], bf16, tag="wsrc")
            nc.vector.memset(wsrc, 0.0)
            wps = psS.tile([128, 512], f32, tag="s0", name="warm_ps")
            for _ in range(10):
                nc.tensor.matmul(
                    out=wps[:, :],
                    lhsT=wsrc[:, 0:128],
                    rhs=wsrc[:, :],
                    start=True,
                    stop=True,
                )

            for pair in range(NPAIR):
                # ---- input loads (sync engine queue) ----
                qtaA = sbq.tile([128, AH], bf16, tag="qA", name=f"qA{pair}")
                ktaA = sbk.tile([128, 16 * BLOCK], bf16, tag="kA", name=f"kA{pair}")
                va_a = [None, None]
                va_b = [None, None]
                nc.sync.dma_start(out=qtaA, in_=qt[pair, :, 0:AH])
                nc.sync.dma_start(out=ktaA, in_=kt[pair, :, 0 : 16 * BLOCK])
                for s in range(2):
                    va_a[s] = sbv.tile(
                        [128, 16 * VW], bf16, tag=f"vA{s}", name=f"vA{s}_{pair}"
                    )
                    nc.sync.dma_start(
                        out=va_a[s], in_=va[pair, s, :, 0 : 16 * VW]
                    )
                qtaB = sbq.tile([128, AH], bf16, tag="qB", name=f"qB{pair}")
                ktaB = sbk.tile([128, 16 * BLOCK], bf16, tag="kB", name=f"kB{pair}")
                nc.sync.dma_start(out=qtaB, in_=qt[pair, :, T - AH : T])
                nc.sync.dma_start(out=ktaB, in_=kt[pair, :, 16 * BLOCK : T])
                for s in range(2):
                    va_b[s] = sbv.tile(
                        [128, 16 * VW], bf16, tag=f"vB{s}", name=f"vB{s}_{pair}"
                    )
                    nc.sync.dma_start(
                        out=va_b[s], in_=va[pair, s, :, 16 * VW : NB * VW]
                    )

                pts = {}     # j -> (pt tile, col base of window pos 0)
                gens = {}    # (s, g) -> psum generation tile
                obts = {}    # (s, t) -> output staging tile

                def qk(j):
                    """scores^T for key block j of both bh (concurrent).

                    Per-bh score banks, pt tiles, and exp instructions keep
                    every dependency half-width: each exp starts as soon as
                    its own bh's matmul drains, and each PV matmul waits only
                    on its own bh's exp. Engines alternate by (j + s) parity
                    so both ACT and DVE run one 384-col exp every j.
                    """
                    # query union = blocks j-1..j+1 clipped; window pos w
                    # covers query block j-1+w at psum cols 128w
                    wlo = 1 if j == 0 else 0
                    whi = 2 if j == NB - 1 else 3
                    if j <= 15:
                        kta, kcol = ktaA, j * BLOCK
                        qta, qbase = qtaA, 0
                    else:
                        kta, kcol = ktaB, (j - 16) * BLOCK
                        qta, qbase = qtaB, T - AH
                    qlo = (j - 1 + wlo) * BLOCK - qbase
                    n = (whi - wlo) * BLOCK
                    ptj = []
                    for s in range(2):
                        p0, p1 = (0, 64) if s == 0 else (64, 128)
                        sP = psS.tile(
                            [128, 512], f32, tag=f"s{s}", name=f"sP{pair}_{j}_{s}"
                        )
                        nc.tensor.matmul(
                            out=sP[:, wlo * BLOCK : wlo * BLOCK + n],
                            lhsT=kta[p0:p1, kcol : kcol + BLOCK],
                            rhs=qta[p0:p1, qlo : qlo + n],
                            start=True,
                            stop=True,
                        )
                        pt = sbp.tile(
                            [128, 384], bf16, tag=f"pt{s}", name=f"pt{pair}_{j}_{s}"
                        )
                        s_view = sP[:, wlo * BLOCK : whi * BLOCK]
                        p_view = pt[:, wlo * BLOCK : whi * BLOCK]
                        if (j + s) % 2 == 0:
                            nc.scalar.activation(
                                out=p_view,
                                in_=s_view,
                                func=mybir.ActivationFunctionType.Exp,
                                bias=bias_tile[:, :],
                                scale=SCALE,
                            )
                        else:
                            nc.vector.tensor_scalar(
                                out=p_view.bitcast(i16),
                                in0=s_view,
                                scalar1=SCH_A,
                                scalar2=SCH_B,
                                op0=mybir.AluOpType.mult,
                                op1=mybir.AluOpType.add,
                            )
                        ptj.append(pt)
                    pts[j] = ptj

                def pv(j):
                    """PV matmuls consuming pt_j: query blocks j-1..j+1."""
                    for s in range(2):
                        ptj = pts[j][s]
                        vaa = (va_a[s], va_b[s])[j // 16]
                        vcol = (j % 16) * VW
                        for qb in range(max(0, j - 1), min(NB - 1, j + 1) + 1):
                            g = qb // 4
                            key = (s, g)
                            if key not in gens:
                                gens[key] = psG.tile(
                                    [128, 512],
                                    f32,
                                    tag=f"g{s}",
                                    name=f"gen{pair}_{s}_{g}",
                                )
                            gt = gens[key]
                            # bank-generation first write: (j, qb) == (4g-1, 4g)
                            # (or (0, 0) for g == 0)
                            st = qb == 4 * g and j == max(0, 4 * g - 1)
                            # bank-generation last write: (4g+4, 4g+3) / (31, 31)
                            sp = qb == 4 * g + 3 and j == min(NB - 1, 4 * g + 4)
                            w = qb - (j - 1)  # window position 0..2
                            c = w * BLOCK
                            o = GSTRIDE * (qb % 4)
                            nc.tensor.matmul(
                                out=gt[:, o : o + VW],
                                lhsT=ptj[:, c : c + BLOCK],
                                rhs=vaa[:, vcol : vcol + VW],
                                start=st,
                                stop=sp,
                                skip_group_check=True,
                            )

                def close_gen(s, g):
                    """Copy finished generation bank to staging; DMA per 2.

                    bh slot 0's copies ride the scalar engine, slot 1's the
                    vector engine, so neither engine eats the full copy load
                    on top of its exp share.
                    """
                    gt = gens.pop((s, g))
                    t = g // 2
                    key = (s, t)
                    if key not in obts:
                        obts[key] = sbo.tile(
                            [128, 2 * 4 * VW],
                            bf16,
                            tag=f"ob{s}",
                            name=f"ob{pair}_{s}_{t}",
                        )
                    obt = obts[key]
                    dst = obt.rearrange("p (b w) -> p b w", w=VW)[
                        :, 4 * (g % 2) : 4 * (g % 2) + 4, :
                    ]
                    src = gt[:, 0 : 4 * GSTRIDE].rearrange(
                        "p (b w) -> p b w", w=GSTRIDE
                    )[:, :, 0:VW]
                    if s == 0:
                        nc.scalar.copy(out=dst, in_=src)
                    else:
                        nc.vector.tensor_copy(out=dst, in_=src)
                    if g % 2 == 1:
                        obts.pop(key)
                        nc.gpsimd.dma_start(out=out[pair, s, t], in_=obt)

                # PV trails the scores by 2 key blocks so the PE never waits
                # on the exp that was just issued; copies trail their last
                # write by the same slack.
                for j in range(NB):
                    qk(j)
                    if j >= 2:
                        pv(j - 2)
                        for s in range(2):
                            # generation g's last write lands at pv(4g+4)
                            if (j - 2) >= 4 and (j - 2) % 4 == 0:
                                close_gen(s, (j - 2 - 4) // 4)
                pv(NB - 2)
                pv(NB - 1)
                for s in range(2):
                    close_gen(s, 7)
                pts.clear()
    nc.compile()
    return nc


def _host_tensors(q, k, v):
    """Build device input arrays from [BH, T, D] fp32 q/k/v.

    qt/kt [BH//2, 128, T]: d-major transposes, bh even on partitions 0-63,
        bh odd on 64-127.
    va [BH//2, 2, 128, NB*65]: per key block j, cols 65j..65j+64 hold
        [v[128j + p, :], 1.0] on partition p.
    """
    qtT = np.ascontiguousarray(q.transpose(0, 2, 1)).astype(_BF16)  # [BH, 64, T]
    ktT = np.ascontiguousarray(k.transpose(0, 2, 1)).astype(_BF16)
    qt = qtT.reshape(BH // 2, 128, T)
    kt = ktT.reshape(BH // 2, 128, T)

    va = np.empty((BH, 128, NB, VW), dtype=_BF16)
    va[:, :, :, :D] = v.reshape(BH, NB, BLOCK, D).transpose(0, 2, 1, 3)
    va[:, :, :, D] = np.float32(1.0)
    va = va.reshape(BH // 2, 2, 128, NB * VW)
    return qt, kt, va


def _in_maps(qt, kt, va):
    maps = []
    for c in range(NCORES):
        s = slice(c * NPAIR, (c + 1) * NPAIR)
        maps.append({"qt": qt[s], "kt": kt[s], "va": va[s]})
    return maps


def _epilogue(outT, q, k, v, mask):
    """outT: [BH//2, 2, 4, 128, 520] bf16 device result -> [N,H,T,D] f32."""
    # -> [BH, tile4, gen2, qb4, 128, 65] -> [BH, 32 qb, 128, 65]
    o = outT.astype(np.float32).reshape(BH, 4, 128, 2, 4, VW)
    o = o.transpose(0, 1, 3, 4, 2, 5).reshape(BH, NB * BLOCK, VW)
    sums = np.ascontiguousarray(o[:, :, D])      # [BH, T]
    o = np.ascontiguousarray(o[:, :, 0:D])       # [BH, T, D]

    # BOS-token key slot for query blocks >= 2 (blocks 0/1 already have key 0
    # inside their local window, which equals the reference's global slot).
    k0 = k[:, 0, :]
    v0 = v[:, 0, :]
    qs = q[:, 2 * BLOCK :, :]
    pk = np.exp(np.einsum("bqd,bd->bq", qs, k0) * SCALE)
    o[:, 2 * BLOCK :, :] += pk[:, :, None] * v0[:, None, :]
    sums[:, 2 * BLOCK :] += pk

    o /= sums[:, :, None]

    # BOS query row: full attention of query 0 over all T keys.
    mrow = np.repeat(mask[:, 0, 0, :], H, axis=0)  # [BH, T]
    s0 = np.einsum("bd,btd->bt", q[:, 0, :], k) * SCALE + mrow
    s0 -= s0.max(axis=1, keepdims=True)
    p0 = np.exp(s0)
    p0 /= p0.sum(axis=1, keepdims=True)
    o[:, 0, :] = np.einsum("bt,btd->bd", p0, v)

    return o.reshape(N, H, T, D).astype(np.float32)


def kernel(query_layer, key_layer, value_layer, attention_mask):
    from concourse.bass_utils import run_bass_kernel_spmd

    q = np.asarray(query_layer, dtype=np.float32).reshape(BH, T, D)
    k = np.asarray(key_layer, dtype=np.float32).reshape(BH, T, D)
    v = np.asarray(value_layer, dtype=np.float32).reshape(BH, T, D)
    mask = np.asarray(attention_mask, dtype=np.float32)  # [N,1,1,T]

    qt, kt, va = _host_tensors(q, k, v)

    if "nc" not in _CACHE:
        _CACHE["nc"] = _build_bass()
    nc = _CACHE["nc"]

    res = run_bass_kernel_spmd(nc, _in_maps(qt, kt, va), core_ids=list(range(NCORES)))
    outT = np.concatenate([np.asarray(r["out"]) for r in res.results], axis=0)
    return _epilogue(outT, q, k, v, mask)


# revision 13
# speedup vs baseline: 1.0809x; 1.0809x over previous
"""LSG (local-sparse-global) block-local self-attention for Trainium2.

Problem: n=2, h=16, t=4096, d=64, block=128. Each query block attends to a
3-block local key window (1-block halo each side) plus a global BOS token
slot; the BOS query (position 0) attends to everything.

Strategy (8 NeuronCores, batch*head = 32 sharded 4 per core, processed
sequentially per core):

  - QK^T: key blocks ride the two PE row groups row-paired (even key
    blocks' d-major kT on partitions 0-63, odd on 64-127), each matmul
    streaming its own 384-wide query union into its own PSUM bank, so the
    two matmuls of a step run concurrently on the systolic array. qT is
    duplicated onto both partition halves on-device (SBUF->SBUF DMA) to
    feed both row groups without extra HBM traffic.
  - softmax has no running max: p = exp(s/8). Scores/8 are ~N(0,1) so
    plain exp stays in range and constant bias cancels after the host
    normalization. Exp alternates per step between ACT (exact spline Exp)
    and DVE (single-pass Schraudolph: i16 = round(s*23.083 + b) written
    through an int16 bitcast IS bf16 exp(s/8), +-3% sawtooth that mostly
    cancels after normalization).
  - PV runs transposed so M=128 (full PE array): out[q, 0:65] accumulates
    pt_j^T @ [v_j | 1] with pt (keys x queries) stationary and the
    65-wide v-augmented block moving. Accumulation rides per-element
    has_written PSUM semantics in per-4-query-block "generation" banks:
    one start=True on the bank's first write per generation, everything
    else accumulates or first-writes naturally.
  - Only one bh is active at a time, so generations need just 2 PSUM
    banks and the score ring gets depth 3 (6 banks) - deep enough that
    the exp latency never gates the next score matmul.
  - outputs (64 dims + softmax sum) leave as bf16; host divides by sums,
    adds the BOS-token key slot for query blocks >= 2 (blocks 0/1 already
    have key 0 in-window, matching the reference's global-slot
    semantics), and computes the single BOS query row (~0.5% of FLOPs).
"""

import sys

import numpy as np
import ml_dtypes

try:  # concourse (bass) ships in the trn_rl repo, not on the default path
    import concourse.bass  # noqa: F401
except ImportError:
    for _p in ("/opt/trn_rl_repo", "/root/.axon_site/_ro/trn_rl_repo"):
        if _p not in sys.path:
            sys.path.insert(0, _p)

N, H, T, D = 2, 16, 4096, 64
BLOCK = 128
NB = T // BLOCK            # 32 key/query blocks
NP = NB // 2               # 16 key-block pairs (steps)
BH = N * H                 # 32 batch*head pairs
NCORES = 8
BH_PER_CORE = BH // NCORES  # 4
SCALE = 1.0 / 8.0           # 1/sqrt(64)
VW = D + 1                  # 65: v columns + ones column
GSTRIDE = 66                # psum column stride per query-block group (8B align)

SCH_C = 0.02
SCH_A = float(128.0 * np.log2(np.e) * SCALE)     # 23.083
SCH_B = float(16256.0 - SCH_C * 128.0)

_BF16 = ml_dtypes.bfloat16

_CACHE = {}


def _build_bass():
    import concourse.bacc as bacc
    import concourse.mybir as mybir
    import concourse.tile as tile

    bf16 = mybir.dt.bfloat16
    i16 = mybir.dt.int16
    f32 = mybir.dt.float32

    nc = bacc.Bacc(None, target_bir_lowering=False)
    # qt: [bh, 64, T] d-major transpose (single copy; device duplicates).
    qt = nc.declare_dram_parameter("qt", [BH_PER_CORE, 64, T], bf16, isOutput=False)
    # kt: [bh, 128, NP*128] row-paired kT: [0:64, 128p:...] = block 2p,
    # [64:128, ...] = block 2p+1.
    kt = nc.declare_dram_parameter(
        "kt", [BH_PER_CORE, 128, NP * BLOCK], bf16, isOutput=False
    )
    # va: [bh, 128, NB*65]: per key block j, cols 65j..65j+64 hold
    # [v[128j + p, :], 1.0] on partition p.
    va = nc.declare_dram_parameter(
        "va", [BH_PER_CORE, 128, NB * VW], bf16, isOutput=False
    )
    # out: [bh, 4, 128, 520]: tile t holds query blocks 8t..8t+7 as
    # 8 x 65 col groups; partition dim = query within block.
    out = nc.declare_dram_parameter(
        "out", [BH_PER_CORE, 4, 128, 2 * 4 * VW], bf16, isOutput=True
    )

    AH = 17 * BLOCK     # 2176: A-half q tile width (blocks 0-16)

    with tile.TileContext(nc) as tc:
        with (
            tc.tile_pool(name="cst", bufs=1) as cst,
            tc.tile_pool(name="sbq", bufs=1) as sbq,
            tc.tile_pool(name="sbk", bufs=1) as sbk,
            tc.tile_pool(name="sbv", bufs=1) as sbv,
            tc.tile_pool(name="sbp", bufs=4) as sbp,
            tc.tile_pool(name="sbo", bufs=2) as sbo,
            tc.tile_pool(name="psS", bufs=3, space="PSUM") as psS,
            tc.tile_pool(name="psG", bufs=2, space="PSUM") as psG,
        ):
            bias_tile = cst.tile([128, 1], f32, tag="bias")
            nc.vector.memset(bias_tile, 0.0)
            # Touch Exp from ACT once: loads the exp table set during warmup
            # and keeps later Exp ops from each carrying a cross-engine wait.
            warm = cst.tile([128, 1], f32, tag="warm")
            nc.scalar.activation(
                out=warm,
                in_=bias_tile,
                func=mybir.ActivationFunctionType.Exp,
                bias=0.0,
                scale=1.0,
            )
            # PE warmup: back-to-back matmuls while the first loads run.
            wsrc = cst.tile([64, 512], bf16, tag="wsrc")
            nc.vector.memset(wsrc, 0.0)
            wps = psS.tile([128, 1024], f32, tag="sp", name="warm_ps")
            for _ in range(10):
                nc.tensor.matmul(
                    out=wps[:, 0:512],
                    lhsT=wsrc[:, 0:128],
                    rhs=wsrc[:, :],
                    start=True,
                    stop=True,
                )

            # ---- input loads. bh 0 gets small lead-in chunks so its first
            # steps start early; q halves are duplicated onto partitions
            # 64-127 via SBUF->SBUF DMA on the gpsimd queue.
            tiles = []          # per bh: dict of tiles
            for bh in range(BH_PER_CORE):
                d = {}
                if bh == 0:
                    d["q0"] = sbq.tile([128, 5 * BLOCK], bf16, tag="q0", name="q0")
                    d["k0"] = sbk.tile([128, 2 * BLOCK], bf16, tag="k0", name="k0")
                    d["v0"] = sbv.tile([128, 4 * VW], bf16, tag="v0", name="v0")
                    nc.sync.dma_start(out=d["q0"][0:64, :], in_=qt[0, :, 0 : 5 * BLOCK])
                    nc.sync.dma_start(out=d["k0"], in_=kt[0, :, 0 : 2 * BLOCK])
                    nc.sync.dma_start(out=d["v0"], in_=va[0, :, 0 : 4 * VW])
                    nc.gpsimd.dma_start(
                        out=d["q0"][64:128, :], in_=d["q0"][0:64, :]
                    )
                    # qA covers blocks 3-16 for steps 2-7; kA pairs 2-7.
                    d["qA"] = sbq.tile(
                        [128, 14 * BLOCK], bf16, tag=f"qA{bh}", name=f"qA{bh}"
                    )
                    d["qA_base"] = 3 * BLOCK
                    d["kA"] = sbk.tile(
                        [128, 6 * BLOCK], bf16, tag=f"kA{bh}", name=f"kA{bh}"
                    )
                    d["kA_base"] = 2
                    d["vA"] = sbv.tile(
                        [128, 12 * VW], bf16, tag=f"vA{bh}", name=f"vA{bh}"
                    )
                    d["vA_base"] = 4
                    nc.sync.dma_start(
                        out=d["qA"][0:64, :], in_=qt[0, :, 3 * BLOCK : 17 * BLOCK]
                    )
                    nc.sync.dma_start(
                        out=d["kA"], in_=kt[0, :, 2 * BLOCK : 8 * BLOCK]
                    )
                    nc.sync.dma_start(out=d["vA"], in_=va[0, :, 4 * VW : 16 * VW])
                    nc.gpsimd.dma_start(
                        out=d["qA"][64:128, :], in_=d["qA"][0:64, :]
                    )
                else:
                    d["qA"] = sbq.tile([128, AH], bf16, tag=f"qA{bh}", name=f"qA{bh}")
                    d["qA_base"] = 0
                    d["kA"] = sbk.tile(
                        [128, 8 * BLOCK], bf16, tag=f"kA{bh}", name=f"kA{bh}"
                    )
                    d["kA_base"] = 0
                    d["vA"] = sbv.tile([128, 16 * VW], bf16, tag=f"vA{bh}", name=f"vA{bh}")
                    d["vA_base"] = 0
                    nc.sync.dma_start(out=d["qA"][0:64, :], in_=qt[bh, :, 0:AH])
                    nc.sync.dma_start(out=d["kA"], in_=kt[bh, :, 0 : 8 * BLOCK])
                    nc.sync.dma_start(out=d["vA"], in_=va[bh, :, 0 : 16 * VW])
                    nc.gpsimd.dma_start(out=d["qA"][64:128, :], in_=d["qA"][0:64, :])
                d["qB"] = sbq.tile([128, AH], bf16, tag=f"qB{bh}", name=f"qB{bh}")
                d["kB"] = sbk.tile([128, 8 * BLOCK], bf16, tag=f"kB{bh}", name=f"kB{bh}")
                d["vB"] = sbv.tile([128, 16 * VW], bf16, tag=f"vB{bh}", name=f"vB{bh}")
                nc.sync.dma_start(out=d["qB"][0:64, :], in_=qt[bh, :, T - AH : T])
                nc.sync.dma_start(out=d["kB"], in_=kt[bh, :, 8 * BLOCK : NP * BLOCK])
                nc.sync.dma_start(out=d["vB"], in_=va[bh, :, 16 * VW : NB * VW])
                nc.gpsimd.dma_start(out=d["qB"][64:128, :], in_=d["qB"][0:64, :])
                tiles.append(d)

            for bh in range(BH_PER_CORE):
                d = tiles[bh]
                pts = {}     # step p -> pt tile [128, 768]
                gens = {}    # g -> psum generation tile
                obts = {}    # t -> output staging tile

                def qsel(j):
                    if bh == 0 and j <= 3:
                        return tiles[0]["q0"], 0
                    if j <= 15:
                        return d["qA"], d["qA_base"]
                    return d["qB"], T - AH

                def ksel(p):
                    if bh == 0 and p <= 1:
                        return tiles[0]["k0"], 0
                    if p <= 7:
                        return d["kA"], d["kA_base"]
                    return d["kB"], 8

                def vsel(j):
                    if bh == 0 and j <= 3:
                        return tiles[0]["v0"], 0
                    if j <= 15:
                        return d["vA"], d["vA_base"]
                    return d["vB"], 16

                def qk(p):
                    """scores^T for key blocks 2p (rows 0-63) and 2p+1
                    (rows 64-127), concurrent row-tiled matmuls."""
                    sP = psS.tile([128, 1024], f32, tag="sp", name=f"sP{bh}_{p}")
                    kta, kbase = ksel(p)
                    for sub in range(2):
                        j = 2 * p + sub
                        wlo = 1 if j == 0 else 0
                        whi = 2 if j == NB - 1 else 3
                        qta, qbase = qsel(j)
                        qlo = (j - 1 + wlo) * BLOCK - qbase
                        n = (whi - wlo) * BLOCK
                        p0 = 64 * sub
                        nc.tensor.matmul(
                            out=sP[:, 512 * sub + wlo * BLOCK : 512 * sub + wlo * BLOCK + n],
                            lhsT=kta[p0 : p0 + 64, (p - kbase) * BLOCK : (p - kbase + 1) * BLOCK],
                            rhs=qta[p0 : p0 + 64, qlo : qlo + n],
                            start=True,
                            stop=True,
                        )
                    ptp = sbp.tile([128, 768], bf16, tag="pt", name=f"pt{bh}_{p}")
                    use_act = (bh + p) % 2 == 0

                    def emit_exp(s_view, p_view, act):
                        if act:
                            nc.scalar.activation(
                                out=p_view,
                                in_=s_view,
                                func=mybir.ActivationFunctionType.Exp,
                                bias=bias_tile[:, :],
                                scale=SCALE,
                            )
                        else:
                            nc.vector.tensor_scalar(
                                out=p_view.bitcast(i16),
                                in0=s_view,
                                scalar1=SCH_A,
                                scalar2=SCH_B,
                                op0=mybir.AluOpType.mult,
                                op1=mybir.AluOpType.add,
                            )

                    if p == 0 or p == NP - 1:
                        # edge subs have different valid windows: two exps,
                        # one per engine so they run in parallel
                        for sub in range(2):
                            j = 2 * p + sub
                            wlo = 1 if j == 0 else 0
                            whi = 2 if j == NB - 1 else 3
                            emit_exp(
                                sP[:, 512 * sub + wlo * BLOCK : 512 * sub + whi * BLOCK],
                                ptp[:, 384 * sub + wlo * BLOCK : 384 * sub + whi * BLOCK],
                                use_act if sub == 0 else not use_act,
                            )
                    else:
                        emit_exp(
                            sP.rearrange("q (b w) -> q b w", b=2)[:, :, 0:384],
                            ptp.rearrange("q (b w) -> q b w", b=2)[:, :, 0:384],
                            use_act,
                        )
                    pts[p] = ptp

                def pv(j):
                    """PV matmuls consuming key block j: query blocks
                    j-1..j+1 accumulate pt_j^T @ [v_j | 1]."""
                    ptp = pts[j // 2]
                    base = 384 * (j % 2)
                    vaa, vbase = vsel(j)
                    vcol = (j - vbase) * VW
                    for qb in range(max(0, j - 1), min(NB - 1, j + 1) + 1):
                        g = qb // 4
                        if g not in gens:
                            gens[g] = psG.tile(
                                [128, 512], f32, tag="g", name=f"gen{bh}_{g}"
                            )
                        gt = gens[g]
                        st = qb == 4 * g and j == max(0, 4 * g - 1)
                        sp = qb == 4 * g + 3 and j == min(NB - 1, 4 * g + 4)
                        w = qb - (j - 1)
                        c = base + w * BLOCK
                        o = GSTRIDE * (qb % 4)
                        nc.tensor.matmul(
                            out=gt[:, o : o + VW],
                            lhsT=ptp[:, c : c + BLOCK],
                            rhs=vaa[:, vcol : vcol + VW],
                            start=st,
                            stop=sp,
                            skip_group_check=True,
                        )

                def close_gen(g):
                    """Copy finished generation bank to staging; DMA per 2.
                    Copy engine alternates so neither ACT nor DVE eats the
                    full copy load on top of its exp share."""
                    gt = gens.pop(g)
                    t = g // 2
                    if t not in obts:
                        obts[t] = sbo.tile(
                            [128, 2 * 4 * VW], bf16, tag="ob", name=f"ob{bh}_{t}"
                        )
                    obt = obts[t]
                    dst = obt.rearrange("p (b w) -> p b w", w=VW)[
                        :, 4 * (g % 2) : 4 * (g % 2) + 4, :
                    ]
                    src = gt[:, 0 : 4 * GSTRIDE].rearrange(
                        "p (b w) -> p b w", w=GSTRIDE
                    )[:, :, 0:VW]
                    if (bh + g) % 2 == 0:
                        nc.scalar.copy(out=dst, in_=src)
                    else:
                        nc.vector.tensor_copy(out=dst, in_=src)
                    if g % 2 == 1:
                        obts.pop(t)
                        nc.gpsimd.dma_start(out=out[bh, t], in_=obt)

                # PV trails the scores by one step (2 key blocks) so the PE
                # never waits on the exp that was just issued.
                for p in range(NP):
                    qk(p)
                    if p >= 1:
                        pv(2 * p - 2)
                        pv(2 * p - 1)
                        # gen g's last write is key 4g+4 = 2p-2 at odd p
                        if p >= 3 and p % 2 == 1:
                            close_gen((p - 3) // 2)
                pv(NB - 2)
                pv(NB - 1)
                close_gen(7)
    nc.compile()
    return nc


def _host_tensors(q, k, v):
    """Build device input arrays from [BH, T, D] fp32 q/k/v.

    qt [BH, 64, T]: d-major transpose (single copy).
    kt [BH, 128, NP*128]: row-paired kT (even key block on partitions 0-63,
        odd on 64-127).
    va [BH, 128, NB*65]: per key block j, cols 65j..65j+64 hold
        [v[128j + p, :], 1.0] on partition p.
    """
    qt = np.ascontiguousarray(q.transpose(0, 2, 1)).astype(_BF16)  # [BH, 64, T]
    ktT = np.ascontiguousarray(k.transpose(0, 2, 1)).astype(_BF16)

    ktb = ktT.reshape(BH, 64, NB, BLOCK)
    kt = np.empty((BH, 128, NP * BLOCK), dtype=_BF16)
    kt[:, 0:64] = ktb[:, :, 0::2].reshape(BH, 64, NP * BLOCK)
    kt[:, 64:128] = ktb[:, :, 1::2].reshape(BH, 64, NP * BLOCK)

    va = np.empty((BH, 128, NB, VW), dtype=_BF16)
    va[:, :, :, :D] = v.reshape(BH, NB, BLOCK, D).transpose(0, 2, 1, 3)
    va[:, :, :, D] = np.float32(1.0)
    va = va.reshape(BH, 128, NB * VW)
    return qt, kt, va


def _in_maps(qt, kt, va):
    maps = []
    for c in range(NCORES):
        s = slice(c * BH_PER_CORE, (c + 1) * BH_PER_CORE)
        maps.append({"qt": qt[s], "kt": kt[s], "va": va[s]})
    return maps


def _epilogue(outT, q, k, v, mask):
    """outT: [BH, 4, 128, 520] bf16 device result -> [N,H,T,D] f32."""
    # -> [BH, tile4, 128, gen2, qb4, 65] -> [BH, 32 qb, 128, 65]
    o = outT.astype(np.float32).reshape(BH, 4, 128, 2, 4, VW)
    o = o.transpose(0, 1, 3, 4, 2, 5).reshape(BH, NB * BLOCK, VW)
    sums = np.ascontiguousarray(o[:, :, D])      # [BH, T]
    o = np.ascontiguousarray(o[:, :, 0:D])       # [BH, T, D]

    # BOS-token key slot for query blocks >= 2 (blocks 0/1 already have key 0
    # inside their local window, which equals the reference's global slot).
    k0 = k[:, 0, :]
    v0 = v[:, 0, :]
    qs = q[:, 2 * BLOCK :, :]
    pk = np.exp(np.einsum("bqd,bd->bq", qs, k0) * SCALE)
    o[:, 2 * BLOCK :, :] += pk[:, :, None] * v0[:, None, :]
    sums[:, 2 * BLOCK :] += pk

    o /= sums[:, :, None]

    # BOS query row: full attention of query 0 over all T keys.
    mrow = np.repeat(mask[:, 0, 0, :], H, axis=0)  # [BH, T]
    s0 = np.einsum("bd,btd->bt", q[:, 0, :], k) * SCALE + mrow
    s0 -= s0.max(axis=1, keepdims=True)
    p0 = np.exp(s0)
    p0 /= p0.sum(axis=1, keepdims=True)
    o[:, 0, :] = np.einsum("bt,btd->bd", p0, v)

    return o.reshape(N, H, T, D).astype(np.float32)


def kernel(query_layer, key_layer, value_layer, attention_mask):
    from concourse.bass_utils import run_bass_kernel_spmd

    q = np.asarray(query_layer, dtype=np.float32).reshape(BH, T, D)
    k = np.asarray(key_layer, dtype=np.float32).reshape(BH, T, D)
    v = np.asarray(value_layer, dtype=np.float32).reshape(BH, T, D)
    mask = np.asarray(attention_mask, dtype=np.float32)  # [N,1,1,T]

    qt, kt, va = _host_tensors(q, k, v)

    if "nc" not in _CACHE:
        _CACHE["nc"] = _build_bass()
    nc = _CACHE["nc"]

    res = run_bass_kernel_spmd(nc, _in_maps(qt, kt, va), core_ids=list(range(NCORES)))
    outT = np.concatenate([np.asarray(r["out"]) for r in res.results], axis=0)
    return _epilogue(outT, q, k, v, mask)


# revision 16
# speedup vs baseline: 1.3808x; 1.2775x over previous
"""LSG (local-sparse-global) block-local self-attention for Trainium2.

Problem: n=2, h=16, t=4096, d=64, block=128. Each query block attends to a
3-block local key window (1-block halo each side) plus a global BOS token
slot; the BOS query (position 0) attends to everything.

Strategy (8 NeuronCores, batch*head = 32 sharded 4 per core, processed
sequentially per core):

  - QK^T: key blocks ride the two PE row groups row-paired (even key
    blocks' d-major kT on partitions 0-63, odd on 64-127), each matmul
    streaming its own 384-wide query union into its own PSUM bank, so the
    two matmuls of a step run concurrently on the systolic array. qT is
    duplicated onto both partition halves on-device (SBUF->SBUF DMA) to
    feed both row groups without extra HBM traffic.
  - softmax has no running max: p = exp(s/8). Scores/8 are ~N(0,1) so
    plain exp stays in range and constant bias cancels after the host
    normalization. Exp alternates per step between ACT (exact spline Exp)
    and DVE (single-pass Schraudolph: i16 = round(s*23.083 + b) written
    through an int16 bitcast IS bf16 exp(s/8), +-3% sawtooth that mostly
    cancels after normalization).
  - PV runs transposed so M=128 (full PE array): out[q, 0:65] accumulates
    pt_j^T @ [v_j | 1] with pt (keys x queries) stationary and the
    65-wide v-augmented block moving. Accumulation rides per-element
    has_written PSUM semantics in per-4-query-block "generation" banks:
    one start=True on the bank's first write per generation, everything
    else accumulates or first-writes naturally.
  - Only one bh is active at a time, so generations need just 2 PSUM
    banks and the score ring gets depth 3 (6 banks) - deep enough that
    the exp latency never gates the next score matmul.
  - outputs (64 dims + softmax sum) leave as bf16; host divides by sums,
    adds the BOS-token key slot for query blocks >= 2 (blocks 0/1 already
    have key 0 in-window, matching the reference's global-slot
    semantics), and computes the single BOS query row (~0.5% of FLOPs).
"""

import sys

import numpy as np
import ml_dtypes

try:  # concourse (bass) ships in the trn_rl repo, not on the default path
    import concourse.bass  # noqa: F401
except ImportError:
    for _p in ("/opt/trn_rl_repo", "/root/.axon_site/_ro/trn_rl_repo"):
        if _p not in sys.path:
            sys.path.insert(0, _p)

N, H, T, D = 2, 16, 4096, 64
BLOCK = 128
NB = T // BLOCK            # 32 key/query blocks
NP = NB // 2               # 16 key-block pairs (steps)
BH = N * H                 # 32 batch*head pairs
NCORES = 8
BH_PER_CORE = BH // NCORES  # 4
SCALE = 1.0 / 8.0           # 1/sqrt(64)
VW = D + 1                  # 65: v columns + ones column
GSTRIDE = 66                # psum column stride per query-block group (8B align)

SCH_C = 0.02
SCH_A = float(128.0 * np.log2(np.e) * SCALE)     # 23.083
SCH_B = float(16256.0 - SCH_C * 128.0)

_BF16 = ml_dtypes.bfloat16

_CACHE = {}


def _build_bass():
    import concourse.bacc as bacc
    import concourse.mybir as mybir
    import concourse.tile as tile

    bf16 = mybir.dt.bfloat16
    i16 = mybir.dt.int16
    f32 = mybir.dt.float32

    nc = bacc.Bacc(None, target_bir_lowering=False)
    # qt: [bh, 128, T] d-major transpose duplicated on both partition
    # halves (feeds both PE row groups; duplicated host-side so every DMA
    # is a clean full-width transfer).
    qt = nc.declare_dram_parameter("qt", [BH_PER_CORE, 128, T], bf16, isOutput=False)
    # kt: [bh, 128, NP*128] row-paired kT: [0:64, 128p:...] = block 2p,
    # [64:128, ...] = block 2p+1.
    kt = nc.declare_dram_parameter(
        "kt", [BH_PER_CORE, 128, NP * BLOCK], bf16, isOutput=False
    )
    # va: [bh, 128, NB*65]: per key block j, cols 65j..65j+64 hold
    # [v[128j + p, :], 1.0] on partition p.
    va = nc.declare_dram_parameter(
        "va", [BH_PER_CORE, 128, NB * VW], bf16, isOutput=False
    )
    # out: [bh, 4, 128, 520]: tile t holds query blocks 8t..8t+7 as
    # 8 x 65 col groups; partition dim = query within block.
    out = nc.declare_dram_parameter(
        "out", [BH_PER_CORE, 4, 128, 2 * 4 * VW], bf16, isOutput=True
    )

    AH = 17 * BLOCK     # 2176: A-half q tile width (blocks 0-16)

    with tile.TileContext(nc) as tc:
        with (
            tc.tile_pool(name="cst", bufs=1) as cst,
            tc.tile_pool(name="sbq", bufs=1) as sbq,
            tc.tile_pool(name="sbk", bufs=1) as sbk,
            tc.tile_pool(name="sbv", bufs=1) as sbv,
            tc.tile_pool(name="sbp", bufs=4) as sbp,
            tc.tile_pool(name="sbo", bufs=2) as sbo,
            tc.tile_pool(name="psS", bufs=3, space="PSUM") as psS,
            tc.tile_pool(name="psG", bufs=2, space="PSUM") as psG,
        ):
            bias_tile = cst.tile([128, 1], f32, tag="bias")
            nc.vector.memset(bias_tile, 0.0)
            # Touch Exp from ACT once: loads the exp table set during warmup
            # and keeps later Exp ops from each carrying a cross-engine wait.
            warm = cst.tile([128, 1], f32, tag="warm")
            nc.scalar.activation(
                out=warm,
                in_=bias_tile,
                func=mybir.ActivationFunctionType.Exp,
                bias=0.0,
                scale=1.0,
            )
            # PE warmup: back-to-back matmuls while the first loads run.
            wsrc = cst.tile([64, 512], bf16, tag="wsrc")
            nc.vector.memset(wsrc, 0.0)
            wps = psS.tile([128, 1024], f32, tag="sp", name="warm_ps")
            for _ in range(10):
                nc.tensor.matmul(
                    out=wps[:, 0:512],
                    lhsT=wsrc[:, 0:128],
                    rhs=wsrc[:, :],
                    start=True,
                    stop=True,
                )

            # ---- input loads. bh 0 gets small lead-in chunks so its first
            # steps start early; q halves are duplicated onto partitions
            # 64-127 via SBUF->SBUF DMA on the gpsimd queue.
            tiles = []          # per bh: dict of tiles
            for bh in range(BH_PER_CORE):
                d = {}
                if bh == 0:
                    d["q0"] = sbq.tile([128, 5 * BLOCK], bf16, tag="q0", name="q0")
                    d["k0"] = sbk.tile([128, 2 * BLOCK], bf16, tag="k0", name="k0")
                    d["v0"] = sbv.tile([128, 4 * VW], bf16, tag="v0", name="v0")
                    nc.sync.dma_start(out=d["q0"], in_=qt[0, :, 0 : 5 * BLOCK])
                    nc.sync.dma_start(out=d["k0"], in_=kt[0, :, 0 : 2 * BLOCK])
                    nc.sync.dma_start(out=d["v0"], in_=va[0, :, 0 : 4 * VW])
                    # qA covers blocks 3-16 for steps 2-7; kA pairs 2-7.
                    d["qA"] = sbq.tile(
                        [128, 14 * BLOCK], bf16, tag=f"qA{bh}", name=f"qA{bh}"
                    )
                    d["qA_base"] = 3 * BLOCK
                    d["kA"] = sbk.tile(
                        [128, 6 * BLOCK], bf16, tag=f"kA{bh}", name=f"kA{bh}"
                    )
                    d["kA_base"] = 2
                    d["vA"] = sbv.tile(
                        [128, 12 * VW], bf16, tag=f"vA{bh}", name=f"vA{bh}"
                    )
                    d["vA_base"] = 4
                    nc.sync.dma_start(
                        out=d["qA"], in_=qt[0, :, 3 * BLOCK : 17 * BLOCK]
                    )
                    nc.sync.dma_start(
                        out=d["kA"], in_=kt[0, :, 2 * BLOCK : 8 * BLOCK]
                    )
                    nc.sync.dma_start(out=d["vA"], in_=va[0, :, 4 * VW : 16 * VW])
                else:
                    d["qA"] = sbq.tile([128, AH], bf16, tag=f"qA{bh}", name=f"qA{bh}")
                    d["qA_base"] = 0
                    d["kA"] = sbk.tile(
                        [128, 8 * BLOCK], bf16, tag=f"kA{bh}", name=f"kA{bh}"
                    )
                    d["kA_base"] = 0
                    d["vA"] = sbv.tile([128, 16 * VW], bf16, tag=f"vA{bh}", name=f"vA{bh}")
                    d["vA_base"] = 0
                    nc.sync.dma_start(out=d["qA"], in_=qt[bh, :, 0:AH])
                    nc.sync.dma_start(out=d["kA"], in_=kt[bh, :, 0 : 8 * BLOCK])
                    nc.sync.dma_start(out=d["vA"], in_=va[bh, :, 0 : 16 * VW])
                d["qB"] = sbq.tile([128, AH], bf16, tag=f"qB{bh}", name=f"qB{bh}")
                d["kB"] = sbk.tile([128, 8 * BLOCK], bf16, tag=f"kB{bh}", name=f"kB{bh}")
                d["vB"] = sbv.tile([128, 16 * VW], bf16, tag=f"vB{bh}", name=f"vB{bh}")
                nc.sync.dma_start(out=d["qB"], in_=qt[bh, :, T - AH : T])
                nc.sync.dma_start(out=d["kB"], in_=kt[bh, :, 8 * BLOCK : NP * BLOCK])
                nc.sync.dma_start(out=d["vB"], in_=va[bh, :, 16 * VW : NB * VW])
                tiles.append(d)

            for bh in range(BH_PER_CORE):
                d = tiles[bh]
                pts = {}     # step p -> pt tile [128, 768]
                gens = {}    # g -> psum generation tile
                obts = {}    # t -> output staging tile

                def qsel(j):
                    if bh == 0 and j <= 3:
                        return tiles[0]["q0"], 0
                    if j <= 15:
                        return d["qA"], d["qA_base"]
                    return d["qB"], T - AH

                def ksel(p):
                    if bh == 0 and p <= 1:
                        return tiles[0]["k0"], 0
                    if p <= 7:
                        return d["kA"], d["kA_base"]
                    return d["kB"], 8

                def vsel(j):
                    if bh == 0 and j <= 3:
                        return tiles[0]["v0"], 0
                    if j <= 15:
                        return d["vA"], d["vA_base"]
                    return d["vB"], 16

                def qk(p):
                    """scores^T for key blocks 2p (rows 0-63) and 2p+1
                    (rows 64-127), concurrent row-tiled matmuls."""
                    sP = psS.tile([128, 1024], f32, tag="sp", name=f"sP{bh}_{p}")
                    kta, kbase = ksel(p)
                    for sub in range(2):
                        j = 2 * p + sub
                        wlo = 1 if j == 0 else 0
                        whi = 2 if j == NB - 1 else 3
                        qta, qbase = qsel(j)
                        qlo = (j - 1 + wlo) * BLOCK - qbase
                        n = (whi - wlo) * BLOCK
                        p0 = 64 * sub
                        nc.tensor.matmul(
                            out=sP[:, 512 * sub + wlo * BLOCK : 512 * sub + wlo * BLOCK + n],
                            lhsT=kta[p0 : p0 + 64, (p - kbase) * BLOCK : (p - kbase + 1) * BLOCK],
                            rhs=qta[p0 : p0 + 64, qlo : qlo + n],
                            start=True,
                            stop=True,
                        )
                    ptp = sbp.tile([128, 768], bf16, tag="pt", name=f"pt{bh}_{p}")
                    use_act = (bh + p) % 2 == 0

                    def emit_exp(s_view, p_view, act):
                        if act:
                            nc.scalar.activation(
                                out=p_view,
                                in_=s_view,
                                func=mybir.ActivationFunctionType.Exp,
                                bias=bias_tile[:, :],
                                scale=SCALE,
                            )
                        else:
                            nc.vector.tensor_scalar(
                                out=p_view.bitcast(i16),
                                in0=s_view,
                                scalar1=SCH_A,
                                scalar2=SCH_B,
                                op0=mybir.AluOpType.mult,
                                op1=mybir.AluOpType.add,
                            )

                    if p == 0 or p == NP - 1:
                        # edge subs have different valid windows: two exps,
                        # one per engine so they run in parallel
                        for sub in range(2):
                            j = 2 * p + sub
                            wlo = 1 if j == 0 else 0
                            whi = 2 if j == NB - 1 else 3
                            emit_exp(
                                sP[:, 512 * sub + wlo * BLOCK : 512 * sub + whi * BLOCK],
                                ptp[:, 384 * sub + wlo * BLOCK : 384 * sub + whi * BLOCK],
                                use_act if sub == 0 else not use_act,
                            )
                    else:
                        emit_exp(
                            sP.rearrange("q (b w) -> q b w", b=2)[:, :, 0:384],
                            ptp.rearrange("q (b w) -> q b w", b=2)[:, :, 0:384],
                            use_act,
                        )
                    pts[p] = ptp

                def pv(j):
                    """PV matmuls consuming key block j: query blocks
                    j-1..j+1 accumulate pt_j^T @ [v_j | 1]."""
                    ptp = pts[j // 2]
                    base = 384 * (j % 2)
                    vaa, vbase = vsel(j)
                    vcol = (j - vbase) * VW
                    for qb in range(max(0, j - 1), min(NB - 1, j + 1) + 1):
                        g = qb // 4
                        if g not in gens:
                            gens[g] = psG.tile(
                                [128, 512], f32, tag="g", name=f"gen{bh}_{g}"
                            )
                        gt = gens[g]
                        st = qb == 4 * g and j == max(0, 4 * g - 1)
                        sp = qb == 4 * g + 3 and j == min(NB - 1, 4 * g + 4)
                        w = qb - (j - 1)
                        c = base + w * BLOCK
                        o = GSTRIDE * (qb % 4)
                        nc.tensor.matmul(
                            out=gt[:, o : o + VW],
                            lhsT=ptp[:, c : c + BLOCK],
                            rhs=vaa[:, vcol : vcol + VW],
                            start=st,
                            stop=sp,
                            skip_group_check=True,
                        )

                def close_gen(g):
                    """Copy finished generation bank to staging; DMA per 2.
                    Copy engine alternates so neither ACT nor DVE eats the
                    full copy load on top of its exp share."""
                    gt = gens.pop(g)
                    t = g // 2
                    if t not in obts:
                        obts[t] = sbo.tile(
                            [128, 2 * 4 * VW], bf16, tag="ob", name=f"ob{bh}_{t}"
                        )
                    obt = obts[t]
                    dst = obt.rearrange("p (b w) -> p b w", w=VW)[
                        :, 4 * (g % 2) : 4 * (g % 2) + 4, :
                    ]
                    src = gt[:, 0 : 4 * GSTRIDE].rearrange(
                        "p (b w) -> p b w", w=GSTRIDE
                    )[:, :, 0:VW]
                    if (bh + g) % 2 == 0:
                        nc.scalar.copy(out=dst, in_=src)
                    else:
                        nc.vector.tensor_copy(out=dst, in_=src)
                    if g % 2 == 1:
                        obts.pop(t)
                        nc.gpsimd.dma_start(out=out[bh, t], in_=obt)

                # PV trails the scores by one step (2 key blocks) so the PE
                # never waits on the exp that was just issued.
                for p in range(NP):
                    qk(p)
                    if p >= 1:
                        pv(2 * p - 2)
                        pv(2 * p - 1)
                        # gen g's last write is key 4g+4 = 2p-2 at odd p
                        if p >= 3 and p % 2 == 1:
                            close_gen((p - 3) // 2)
                pv(NB - 2)
                pv(NB - 1)
                close_gen(7)
    nc.compile()
    return nc


def _host_tensors(q, k, v):
    """Build device input arrays from [BH, T, D] fp32 q/k/v.

    qt [BH, 128, T]: d-major transpose duplicated on both partition halves.
    kt [BH, 128, NP*128]: row-paired kT (even key block on partitions 0-63,
        odd on 64-127).
    va [BH, 128, NB*65]: per key block j, cols 65j..65j+64 hold
        [v[128j + p, :], 1.0] on partition p.
    """
    qtT = np.ascontiguousarray(q.transpose(0, 2, 1)).astype(_BF16)  # [BH, 64, T]
    qt = np.empty((BH, 128, T), dtype=_BF16)
    qt[:, 0:64] = qtT
    qt[:, 64:128] = qtT
    ktT = np.ascontiguousarray(k.transpose(0, 2, 1)).astype(_BF16)

    ktb = ktT.reshape(BH, 64, NB, BLOCK)
    kt = np.empty((BH, 128, NP * BLOCK), dtype=_BF16)
    kt[:, 0:64] = ktb[:, :, 0::2].reshape(BH, 64, NP * BLOCK)
    kt[:, 64:128] = ktb[:, :, 1::2].reshape(BH, 64, NP * BLOCK)

    va = np.empty((BH, 128, NB, VW), dtype=_BF16)
    va[:, :, :, :D] = v.reshape(BH, NB, BLOCK, D).transpose(0, 2, 1, 3)
    va[:, :, :, D] = np.float32(1.0)
    va = va.reshape(BH, 128, NB * VW)
    return qt, kt, va


def _in_maps(qt, kt, va):
    maps = []
    for c in range(NCORES):
        s = slice(c * BH_PER_CORE, (c + 1) * BH_PER_CORE)
        maps.append({"qt": qt[s], "kt": kt[s], "va": va[s]})
    return maps


def _epilogue(outT, q, k, v, mask):
    """outT: [BH, 4, 128, 520] bf16 device result -> [N,H,T,D] f32."""
    # -> [BH, tile4, 128, gen2, qb4, 65] -> [BH, 32 qb, 128, 65]
    o = outT.astype(np.float32).reshape(BH, 4, 128, 2, 4, VW)
    o = o.transpose(0, 1, 3, 4, 2, 5).reshape(BH, NB * BLOCK, VW)
    sums = np.ascontiguousarray(o[:, :, D])      # [BH, T]
    o = np.ascontiguousarray(o[:, :, 0:D])       # [BH, T, D]

    # BOS-token key slot for query blocks >= 2 (blocks 0/1 already have key 0
    # inside their local window, which equals the reference's global slot).
    k0 = k[:, 0, :]
    v0 = v[:, 0, :]
    qs = q[:, 2 * BLOCK :, :]
    pk = np.exp(np.einsum("bqd,bd->bq", qs, k0) * SCALE)
    o[:, 2 * BLOCK :, :] += pk[:, :, None] * v0[:, None, :]
    sums[:, 2 * BLOCK :] += pk

    o /= sums[:, :, None]

    # BOS query row: full attention of query 0 over all T keys.
    mrow = np.repeat(mask[:, 0, 0, :], H, axis=0)  # [BH, T]
    s0 = np.einsum("bd,btd->bt", q[:, 0, :], k) * SCALE + mrow
    s0 -= s0.max(axis=1, keepdims=True)
    p0 = np.exp(s0)
    p0 /= p0.sum(axis=1, keepdims=True)
    o[:, 0, :] = np.einsum("bt,btd->bd", p0, v)

    return o.reshape(N, H, T, D).astype(np.float32)


def kernel(query_layer, key_layer, value_layer, attention_mask):
    from concourse.bass_utils import run_bass_kernel_spmd

    q = np.asarray(query_layer, dtype=np.float32).reshape(BH, T, D)
    k = np.asarray(key_layer, dtype=np.float32).reshape(BH, T, D)
    v = np.asarray(value_layer, dtype=np.float32).reshape(BH, T, D)
    mask = np.asarray(attention_mask, dtype=np.float32)  # [N,1,1,T]

    qt, kt, va = _host_tensors(q, k, v)

    if "nc" not in _CACHE:
        _CACHE["nc"] = _build_bass()
    nc = _CACHE["nc"]

    res = run_bass_kernel_spmd(nc, _in_maps(qt, kt, va), core_ids=list(range(NCORES)))
    outT = np.concatenate([np.asarray(r["out"]) for r in res.results], axis=0)
    return _epilogue(outT, q, k, v, mask)


# revision 17
# speedup vs baseline: 1.4130x; 1.0233x over previous
"""LSG (local-sparse-global) block-local self-attention for Trainium2.

Problem: n=2, h=16, t=4096, d=64, block=128. Each query block attends to a
3-block local key window (1-block halo each side) plus a global BOS token
slot; the BOS query (position 0) attends to everything.

Strategy (8 NeuronCores, batch*head = 32 sharded 4 per core, as 2 "bh
pairs" per core):

  - QK^T: the two bh of a pair ride the two PE row groups (contraction
    d=64 each) concurrently: lhsT = kT(bh0) on partitions 0-63 and
    kT(bh1) on 64-127, each against its own 384-wide query union, into
    separate PSUM banks of one score tile. Full-array utilization, no q
    duplication, and the shared tile keeps the Tile scheduler from
    splitting the pair.
  - softmax has no running max: p = exp(s/8). Scores/8 are ~N(0,1) so
    plain exp stays in range and constant bias cancels after the host
    normalization. Exp alternates per key block between ACT (exact
    spline Exp) and DVE using a single-pass Schraudolph bit-trick:
    i16 = round(s*23.083 + b) written via an int16 bitcast IS the bf16
    encoding of exp(s/8) (+-3% sawtooth, mostly cancelling after
    normalization; ~1e-2 absmax vs the 2e-2 gate).
  - PV runs transposed so M=128 (full array): out[q, 0:65] accumulates
    pt_j^T @ [v_j | 1] with pt (keys x queries) stationary and the
    65-wide v-augmented block moving. Accumulation rides per-element
    has_written PSUM semantics in per-4-query-block "generation" banks:
    one start=True on the bank's first write per generation, everything
    else accumulates or first-writes naturally.
  - PV trails the scores by 2 key blocks so the PE never waits on the
    exp that was just issued; gen copies split across ACT/DVE; outputs
    (64 dims + softmax sum) leave as bf16 via the gpsimd DMA queue.
  - Host divides by sums, adds the BOS-token key slot for query blocks
    >= 2 (blocks 0/1 already have key 0 in-window, matching the
    reference's global-slot semantics), and computes the single BOS
    query row (~0.5% of FLOPs).
"""

import sys

import numpy as np
import ml_dtypes

try:  # concourse (bass) ships in the trn_rl repo, not on the default path
    import concourse.bass  # noqa: F401
except ImportError:
    for _p in ("/opt/trn_rl_repo", "/root/.axon_site/_ro/trn_rl_repo"):
        if _p not in sys.path:
            sys.path.insert(0, _p)

N, H, T, D = 2, 16, 4096, 64
BLOCK = 128
NB = T // BLOCK            # 32 key/query blocks
BH = N * H                 # 32 batch*head pairs
NCORES = 8
BH_PER_CORE = BH // NCORES  # 4
NPAIR = BH_PER_CORE // 2    # 2 bh-pairs per core
SCALE = 1.0 / 8.0           # 1/sqrt(64)
VW = D + 1                  # 65: v columns + ones column
GSTRIDE = 66                # psum column stride per query-block group (8B align)

# exp split: key blocks in DVE_JS run the DVE Schraudolph exp, others ACT
DVE_JS = frozenset(range(0, 32, 2))
SCH_C = 0.02
SCH_A = float(128.0 * np.log2(np.e) * SCALE)     # 23.083
SCH_B = float(16256.0 - SCH_C * 128.0)

_BF16 = ml_dtypes.bfloat16

_CACHE = {}


def _build_bass():
    import concourse.bacc as bacc
    import concourse.mybir as mybir
    import concourse.tile as tile

    bf16 = mybir.dt.bfloat16
    i16 = mybir.dt.int16
    f32 = mybir.dt.float32

    nc = bacc.Bacc(None, target_bir_lowering=False)
    # qt/kt: [pair, 128, T]: partitions 0-63 = bh even (d-major transpose),
    # 64-127 = bh odd.
    qt = nc.declare_dram_parameter("qt", [NPAIR, 128, T], bf16, isOutput=False)
    kt = nc.declare_dram_parameter("kt", [NPAIR, 128, T], bf16, isOutput=False)
    # va: [pair, bh_slot, 128, NB*65]: per key block j, cols 65j..65j+64 hold
    # [v[128j + p, :], 1.0] on partition p.
    va = nc.declare_dram_parameter(
        "va", [NPAIR, 2, 128, NB * VW], bf16, isOutput=False
    )
    # out: [pair, bh_slot, 4, 128, 520]: tile t holds query blocks 8t..8t+7
    # as 8 x 65 col groups; partition dim = query within block.
    out = nc.declare_dram_parameter(
        "out", [NPAIR, 2, 4, 128, 2 * 4 * VW], bf16, isOutput=True
    )

    AH = 17 * BLOCK     # 2176: B-half tile width (blocks 15-31)

    with tile.TileContext(nc) as tc:
        with (
            tc.tile_pool(name="cst", bufs=1) as cst,
            tc.tile_pool(name="sbq", bufs=1) as sbq,
            tc.tile_pool(name="sbk", bufs=1) as sbk,
            tc.tile_pool(name="sbv", bufs=1) as sbv,
            tc.tile_pool(name="sbp", bufs=5) as sbp,
            tc.tile_pool(name="sbo", bufs=2) as sbo,
            tc.tile_pool(name="psS", bufs=2, space="PSUM") as psS,
            tc.tile_pool(name="psG", bufs=2, space="PSUM") as psG,
        ):
            bias_tile = cst.tile([128, 1], f32, tag="bias")
            nc.vector.memset(bias_tile, 0.0)
            # Touch Exp from ACT once: loads the exp table set during warmup
            # and keeps later Exp ops from each carrying a cross-engine wait.
            warm = cst.tile([128, 1], f32, tag="warm")
            nc.scalar.activation(
                out=warm,
                in_=bias_tile,
                func=mybir.ActivationFunctionType.Exp,
                bias=0.0,
                scale=1.0,
            )
            # Short PE warmup: the lead-in chunks land quickly, so just keep
            # the PE pipeline alive until they do.
            wsrc = cst.tile([64, 512], bf16, tag="wsrc")
            nc.vector.memset(wsrc, 0.0)
            wps = psS.tile([128, 1024], f32, tag="spair", name="warm_ps")
            for _ in range(5):
                nc.tensor.matmul(
                    out=wps[:, 0:512],
                    lhsT=wsrc[:, 0:128],
                    rhs=wsrc[:, :],
                    start=True,
                    stop=True,
                )

            # ---- input loads (sync engine queue). Pair 0 gets small
            # lead-in chunks (blocks 0-4) so its first steps start early.
            tiles = []
            for pair in range(NPAIR):
                d = {}
                if pair == 0:
                    d["q0"] = sbq.tile([128, 5 * BLOCK], bf16, tag="q0", name="q0")
                    d["k0"] = sbk.tile([128, 4 * BLOCK], bf16, tag="k0", name="k0")
                    nc.sync.dma_start(out=d["q0"], in_=qt[0, :, 0 : 5 * BLOCK])
                    nc.sync.dma_start(out=d["k0"], in_=kt[0, :, 0 : 4 * BLOCK])
                    d["v0"] = [None, None]
                    for s in range(2):
                        t0 = sbv.tile([128, 4 * VW], bf16, tag=f"v0{s}", name=f"v0{s}")
                        nc.sync.dma_start(out=t0, in_=va[0, s, :, 0 : 4 * VW])
                        d["v0"][s] = t0
                    # A covers blocks 3-16 (q) / 4-15 (k, va) for j in 4..15
                    d["qA"] = sbq.tile(
                        [128, 14 * BLOCK], bf16, tag=f"qA{pair}", name=f"qA{pair}"
                    )
                    d["qA_base"] = 3 * BLOCK
                    d["kA"] = sbk.tile(
                        [128, 12 * BLOCK], bf16, tag=f"kA{pair}", name=f"kA{pair}"
                    )
                    d["kA_base"] = 4
                    nc.sync.dma_start(
                        out=d["qA"], in_=qt[0, :, 3 * BLOCK : 17 * BLOCK]
                    )
                    nc.sync.dma_start(
                        out=d["kA"], in_=kt[0, :, 4 * BLOCK : 16 * BLOCK]
                    )
                    d["vA"] = [None, None]
                    d["vA_base"] = 4
                    for s in range(2):
                        tA = sbv.tile(
                            [128, 12 * VW], bf16, tag=f"vA{pair}{s}", name=f"vA{pair}{s}"
                        )
                        nc.sync.dma_start(out=tA, in_=va[0, s, :, 4 * VW : 16 * VW])
                        d["vA"][s] = tA
                else:
                    d["qA"] = sbq.tile(
                        [128, AH], bf16, tag=f"qA{pair}", name=f"qA{pair}"
                    )
                    d["qA_base"] = 0
                    d["kA"] = sbk.tile(
                        [128, 16 * BLOCK], bf16, tag=f"kA{pair}", name=f"kA{pair}"
                    )
                    d["kA_base"] = 0
                    nc.sync.dma_start(out=d["qA"], in_=qt[pair, :, 0:AH])
                    nc.sync.dma_start(out=d["kA"], in_=kt[pair, :, 0 : 16 * BLOCK])
                    d["vA"] = [None, None]
                    d["vA_base"] = 0
                    for s in range(2):
                        tA = sbv.tile(
                            [128, 16 * VW], bf16, tag=f"vA{pair}{s}", name=f"vA{pair}{s}"
                        )
                        nc.sync.dma_start(out=tA, in_=va[pair, s, :, 0 : 16 * VW])
                        d["vA"][s] = tA
                d["qB"] = sbq.tile([128, AH], bf16, tag=f"qB{pair}", name=f"qB{pair}")
                d["kB"] = sbk.tile(
                    [128, 16 * BLOCK], bf16, tag=f"kB{pair}", name=f"kB{pair}"
                )
                nc.sync.dma_start(out=d["qB"], in_=qt[pair, :, T - AH : T])
                nc.sync.dma_start(out=d["kB"], in_=kt[pair, :, 16 * BLOCK : T])
                d["vB"] = [None, None]
                for s in range(2):
                    tB = sbv.tile(
                        [128, 16 * VW], bf16, tag=f"vB{pair}{s}", name=f"vB{pair}{s}"
                    )
                    nc.sync.dma_start(out=tB, in_=va[pair, s, :, 16 * VW : NB * VW])
                    d["vB"][s] = tB
                tiles.append(d)

            for pair in range(NPAIR):
                d = tiles[pair]
                pts = {}     # j -> pt tile [128, 768]
                gens = {}    # (s, g) -> psum generation tile
                obts = {}    # (s, t) -> output staging tile

                def qsel(j):
                    if pair == 0 and j <= 3:
                        return d["q0"], 0
                    if j <= 15:
                        return d["qA"], d["qA_base"]
                    return d["qB"], T - AH

                def ksel(j):
                    if pair == 0 and j <= 3:
                        return d["k0"], 0
                    if j <= 15:
                        return d["kA"], d["kA_base"]
                    return d["kB"], 16

                def vsel(j, s):
                    if pair == 0 and j <= 3:
                        return d["v0"][s], 0
                    if j <= 15:
                        return d["vA"][s], d["vA_base"]
                    return d["vB"][s], 16

                def qk(j):
                    """scores^T for key block j of both bh (concurrent)."""
                    sP = psS.tile([128, 1024], f32, tag="spair", name=f"sP{pair}_{j}")
                    # query union = blocks j-1..j+1 clipped; window pos w
                    # covers query block j-1+w at psum cols 128w (+512 bh1)
                    wlo = 1 if j == 0 else 0
                    whi = 2 if j == NB - 1 else 3
                    kta, kbase = ksel(j)
                    qta, qbase = qsel(j)
                    kcol = (j - kbase) * BLOCK
                    qlo = (j - 1 + wlo) * BLOCK - qbase
                    n = (whi - wlo) * BLOCK
                    for s in range(2):
                        p0, p1 = (0, 64) if s == 0 else (64, 128)
                        nc.tensor.matmul(
                            out=sP[:, 512 * s + wlo * BLOCK : 512 * s + wlo * BLOCK + n],
                            lhsT=kta[p0:p1, kcol : kcol + BLOCK],
                            rhs=qta[p0:p1, qlo : qlo + n],
                            start=True,
                            stop=True,
                        )
                    # exp into SBUF bf16 pt tile: [0:384]=bh0, [384:768]=bh1
                    ptj = sbp.tile([128, 768], bf16, tag="pt", name=f"pt{pair}_{j}")
                    s_view = sP.rearrange("p (b w) -> p b w", b=2)[
                        :, :, wlo * BLOCK : whi * BLOCK
                    ]
                    p_view = ptj.rearrange("p (b w) -> p b w", b=2)[
                        :, :, wlo * BLOCK : whi * BLOCK
                    ]
                    if j in DVE_JS:
                        nc.vector.tensor_scalar(
                            out=p_view.bitcast(i16),
                            in0=s_view,
                            scalar1=SCH_A,
                            scalar2=SCH_B,
                            op0=mybir.AluOpType.mult,
                            op1=mybir.AluOpType.add,
                        )
                    else:
                        nc.scalar.activation(
                            out=p_view,
                            in_=s_view,
                            func=mybir.ActivationFunctionType.Exp,
                            bias=bias_tile[:, :],
                            scale=SCALE,
                        )
                    pts[j] = ptj

                def pv(j):
                    """PV matmuls consuming pt_j: query blocks j-1..j+1."""
                    ptj = pts[j]
                    for s in range(2):
                        vaa, vbase = vsel(j, s)
                        vcol = (j - vbase) * VW
                        for qb in range(max(0, j - 1), min(NB - 1, j + 1) + 1):
                            g = qb // 4
                            key = (s, g)
                            if key not in gens:
                                gens[key] = psG.tile(
                                    [128, 512],
                                    f32,
                                    tag=f"g{s}",
                                    name=f"gen{pair}_{s}_{g}",
                                )
                            gt = gens[key]
                            # bank-generation first write: (j, qb) == (4g-1, 4g)
                            st = qb == 4 * g and j == max(0, 4 * g - 1)
                            # bank-generation last write: (4g+4, 4g+3) / (31, 31)
                            sp = qb == 4 * g + 3 and j == min(NB - 1, 4 * g + 4)
                            w = qb - (j - 1)  # window position 0..2
                            c = 384 * s + w * BLOCK
                            o = GSTRIDE * (qb % 4)
                            nc.tensor.matmul(
                                out=gt[:, o : o + VW],
                                lhsT=ptj[:, c : c + BLOCK],
                                rhs=vaa[:, vcol : vcol + VW],
                                start=st,
                                stop=sp,
                                skip_group_check=True,
                            )

                def close_gen(s, g):
                    """Copy finished generation bank to staging; DMA per 2.

                    bh slot 0's copies ride the scalar engine, slot 1's the
                    vector engine, so neither engine eats the full copy load
                    on top of its exp share.
                    """
                    gt = gens.pop((s, g))
                    t = g // 2
                    key = (s, t)
                    if key not in obts:
                        obts[key] = sbo.tile(
                            [128, 2 * 4 * VW],
                            bf16,
                            tag=f"ob{s}",
                            name=f"ob{pair}_{s}_{t}",
                        )
                    obt = obts[key]
                    dst = obt.rearrange("p (b w) -> p b w", w=VW)[
                        :, 4 * (g % 2) : 4 * (g % 2) + 4, :
                    ]
                    src = gt[:, 0 : 4 * GSTRIDE].rearrange(
                        "p (b w) -> p b w", w=GSTRIDE
                    )[:, :, 0:VW]
                    if s == 0:
                        nc.scalar.copy(out=dst, in_=src)
                    else:
                        nc.vector.tensor_copy(out=dst, in_=src)
                    if g % 2 == 1:
                        obts.pop(key)
                        nc.gpsimd.dma_start(out=out[pair, s, t], in_=obt)

                # PV trails the scores by 2 key blocks so the PE never waits
                # on the exp that was just issued.
                for j in range(NB):
                    qk(j)
                    if j >= 2:
                        pv(j - 2)
                        for s in range(2):
                            # generation g's last write lands at pv(4g+4)
                            if (j - 2) >= 4 and (j - 2) % 4 == 0:
                                close_gen(s, (j - 2 - 4) // 4)
                pv(NB - 2)
                pv(NB - 1)
                for s in range(2):
                    close_gen(s, 7)
                pts.clear()
    nc.compile()
    return nc


def _host_tensors(q, k, v):
    """Build device input arrays from [BH, T, D] fp32 q/k/v.

    qt/kt [BH//2, 128, T]: d-major transposes, bh even on partitions 0-63,
        bh odd on 64-127.
    va [BH//2, 2, 128, NB*65]: per key block j, cols 65j..65j+64 hold
        [v[128j + p, :], 1.0] on partition p.
    """
    qtT = np.ascontiguousarray(q.transpose(0, 2, 1)).astype(_BF16)  # [BH, 64, T]
    ktT = np.ascontiguousarray(k.transpose(0, 2, 1)).astype(_BF16)
    qt = qtT.reshape(BH // 2, 128, T)
    kt = ktT.reshape(BH // 2, 128, T)

    va = np.empty((BH, 128, NB, VW), dtype=_BF16)
    va[:, :, :, :D] = v.reshape(BH, NB, BLOCK, D).transpose(0, 2, 1, 3)
    va[:, :, :, D] = np.float32(1.0)
    va = va.reshape(BH // 2, 2, 128, NB * VW)
    return qt, kt, va


def _in_maps(qt, kt, va):
    maps = []
    for c in range(NCORES):
        s = slice(c * NPAIR, (c + 1) * NPAIR)
        maps.append({"qt": qt[s], "kt": kt[s], "va": va[s]})
    return maps


def _epilogue(outT, q, k, v, mask):
    """outT: [BH//2, 2, 4, 128, 520] bf16 device result -> [N,H,T,D] f32."""
    # -> [BH, tile4, 128, gen2, qb4, 65] -> [BH, 32 qb, 128, 65]
    o = outT.astype(np.float32).reshape(BH, 4, 128, 2, 4, VW)
    o = o.transpose(0, 1, 3, 4, 2, 5).reshape(BH, NB * BLOCK, VW)
    sums = np.ascontiguousarray(o[:, :, D])      # [BH, T]
    o = np.ascontiguousarray(o[:, :, 0:D])       # [BH, T, D]

    # BOS-token key slot for query blocks >= 2 (blocks 0/1 already have key 0
    # inside their local window, which equals the reference's global slot).
    k0 = k[:, 0, :]
    v0 = v[:, 0, :]
    qs = q[:, 2 * BLOCK :, :]
    pk = np.exp(np.einsum("bqd,bd->bq", qs, k0) * SCALE)
    o[:, 2 * BLOCK :, :] += pk[:, :, None] * v0[:, None, :]
    sums[:, 2 * BLOCK :] += pk

    o /= sums[:, :, None]

    # BOS query row: full attention of query 0 over all T keys.
    mrow = np.repeat(mask[:, 0, 0, :], H, axis=0)  # [BH, T]
    s0 = np.einsum("bd,btd->bt", q[:, 0, :], k) * SCALE + mrow
    s0 -= s0.max(axis=1, keepdims=True)
    p0 = np.exp(s0)
    p0 /= p0.sum(axis=1, keepdims=True)
    o[:, 0, :] = np.einsum("bt,btd->bd", p0, v)

    return o.reshape(N, H, T, D).astype(np.float32)


def kernel(query_layer, key_layer, value_layer, attention_mask):
    from concourse.bass_utils import run_bass_kernel_spmd

    q = np.asarray(query_layer, dtype=np.float32).reshape(BH, T, D)
    k = np.asarray(key_layer, dtype=np.float32).reshape(BH, T, D)
    v = np.asarray(value_layer, dtype=np.float32).reshape(BH, T, D)
    mask = np.asarray(attention_mask, dtype=np.float32)  # [N,1,1,T]

    qt, kt, va = _host_tensors(q, k, v)

    if "nc" not in _CACHE:
        _CACHE["nc"] = _build_bass()
    nc = _CACHE["nc"]

    res = run_bass_kernel_spmd(nc, _in_maps(qt, kt, va), core_ids=list(range(NCORES)))
    outT = np.concatenate([np.asarray(r["out"]) for r in res.results], axis=0)
    return _epilogue(outT, q, k, v, mask)
